# revision 84
# baseline (speedup 1.0000x reference)
"""Trainium2 Bass kernel for sliding-window (+-64) multi-head attention. v2.

Reference computation (seq=4096, hidden=768, 12 heads x 64, RoPE, window 128):
    qkv = qkv_weight @ x ; q,k = rope(q,k) ; scores = q^T k / 8 + band_mask
    attn = softmax(scores) @ v ; out = out_weight @ attn

Sharding: sequence-parallel over 8 cores. Core c owns queries
[512c, 512c+512) and computes K/V over the haloed span [512c-64, 512c+576)
(zero-padded at the sequence edges; phantom keys are killed by per-core edge
band-mask tiles). No collectives; host concatenates the 8 outputs.

v2 structure (vs the v1 baseline):
- QKV projections run as fp8(e4m3) DoubleRow matmuls (256-deep contraction,
  0.5 cyc/row) with 3-term error compensation: W ~ Wh+Wl, X ~ Xh+Xl (scaled
  per-tensor by powers of 2), W@X ~ Wh@Xh + Wl@Xh + Wh@Xl.  The 2^-k
  de-scaling folds into the rope cos/sin tables (Q,K) and the V evacuation
  copy scale, so it costs nothing.
- Scores are computed TRANSPOSED ([key, query] in PSUM, stationary = K), so
  softmax's exp output P^T feeds the PV matmul directly: no PE transpose
  and no PSUM evacuation copy.
- The PV stationary is a strided AP [V_h0(64) | ones(64)] (resp.
  [ones | V_h1]): partitions 0-63 of the PV output accumulate attn for h0
  while 64-127 accumulate the softmax denominator (replicated), and
  mirrored for h1.  Normalization is then a small stride-0 DMA replicating
  the denominator row onto the numerator's partitions plus one DVE divide
  per (head, 512 queries) - no reduce, no reciprocal, no transpose.
- rotate_half runs as a 4-piece SBUF->SBUF DMA partition permutation (sign
  folded into the sin table); rope multiplies are bf16 DVE ops at
  [128, 1152] (Q and K fused per head pair).
"""

import os
import sys

import numpy as np

for _p in ("/opt/trn_rl_repo",):
    if _p not in sys.path and os.path.isdir(_p):
        sys.path.insert(0, _p)

import ml_dtypes

import concourse.bass as bass
import concourse.bacc as bacc
import concourse.tile as tile
from concourse import mybir
from concourse.ap import AP
from concourse.bass_utils import run_bass_kernel_spmd

F32 = mybir.dt.float32
F32R = mybir.dt.float32r
BF16 = mybir.dt.bfloat16
FP8 = mybir.dt.float8e4

N_CORES = 8
SEQ = 4096
S_CORE = SEQ // N_CORES  # 512 queries per core
HALO = 64
SPAN = S_CORE + 2 * HALO  # 640 keys per core
HID = 768
NH = 12
DH = 64
NHP = NH // 2            # 6 head pairs
NSC = SPAN // 128        # 5 key chunks per core
NPR = 3                  # fp8 DoubleRow pair-chunks (3 x 256 = 768)
NQB = S_CORE // 128      # 4 query blocks
QKW = S_CORE + SPAN      # 1152: fused q|k rope width per head pair
VTW = NSC * NHP * 192    # VT tile width: per (kc, hp): [V_h0 | ones | V_h1]

DR = mybir.MatmulPerfMode.DoubleRow

_BUILD_CACHE = {}


def _build(add_mask: bool, isv: int, reps: int = 1):
    nc = bacc.Bacc("TRN2", target_bir_lowering=False, debug=False, num_devices=N_CORES)

    xhl_d = nc.dram_tensor("xhl", [128, 2 * NPR * 2 * SPAN], FP8,
                           kind="ExternalInput")
    wq_d = nc.dram_tensor("wqd", [128, 2 * NHP * NPR * 2 * 128], FP8,
                          kind="ExternalInput")
    wk_d = nc.dram_tensor("wkd", [128, 2 * NHP * NPR * 2 * 128], FP8,
                          kind="ExternalInput")
    wvhl_d = nc.dram_tensor("wvhl", [128, 2 * NPR * 2 * HID], FP8,
                            kind="ExternalInput")
    wot_d = nc.dram_tensor("wot", [128, NHP * 6 * 128], BF16, kind="ExternalInput")
    tabs_d = nc.dram_tensor("tabs", [128, 2 * QKW + 3 * 512], BF16,
                            kind="ExternalInput")
    if add_mask:
        maskf_d = nc.dram_tensor("maskf", [128, NSC * 512], F32, kind="ExternalInput")
    out_d = nc.dram_tensor("out", [128, 6 * S_CORE], BF16, kind="ExternalOutput")

    mult = mybir.AluOpType.mult
    addop = mybir.AluOpType.add
    divop = mybir.AluOpType.divide
    exp = mybir.ActivationFunctionType.Exp
    copyf = mybir.ActivationFunctionType.Copy

    with tile.TileContext(nc) as tc:
        from contextlib import ExitStack

        for _rep in range(reps):
          with ExitStack() as ctx:
            const = ctx.enter_context(tc.tile_pool(name="const", bufs=1))
            sb = ctx.enter_context(tc.tile_pool(name="sb", bufs=1))
            ropep = ctx.enter_context(tc.tile_pool(name="ropep", bufs=2))
            pmp = ctx.enter_context(tc.tile_pool(name="pmp", bufs=6))
            denp = ctx.enter_context(tc.tile_pool(name="denp", bufs=4))
            outp = ctx.enter_context(tc.tile_pool(name="outp", bufs=6))
            ps_proj = ctx.enter_context(
                tc.tile_pool(name="ps_proj", bufs=3, space="PSUM"))
            ps_att = ctx.enter_context(
                tc.tile_pool(name="ps_att", bufs=2, space="PSUM"))
            ps_o = ctx.enter_context(tc.tile_pool(name="ps_o", bufs=3, space="PSUM"))

            # ---- input DMAs (ordered by first use) ----
            XHL = const.tile([128, 2, NPR, 2, SPAN], FP8, tag="XHL")
            nc.sync.dma_start(
                out=XHL[:],
                in_=xhl_d.ap().rearrange(
                    "p (v r t s) -> p v r t s", v=2, r=NPR, t=2))
            XH = XHL[:, 0]
            XL = XHL[:, 1]
            WVHL = const.tile([128, 2, NPR, 2, HID], FP8, tag="WVHL")
            nc.sync.dma_start(
                out=WVHL[:],
                in_=wvhl_d.ap().rearrange(
                    "p (v r t m) -> p v r t m", v=2, r=NPR, t=2))
            WVH = WVHL[:, 0]
            WVL = WVHL[:, 1]
            WQK = const.tile([128, 4, NHP, NPR, 2, 128], FP8, tag="WQK")
            nc.sync.dma_start(
                out=WQK[:, 0:2],
                in_=wq_d.ap().rearrange(
                    "p (v k r t m) -> p v k r t m", v=2, k=NHP, r=NPR, t=2))
            nc.sync.dma_start(
                out=WQK[:, 2:4],
                in_=wk_d.ap().rearrange(
                    "p (v k r t m) -> p v k r t m", v=2, k=NHP, r=NPR, t=2))
            WQH, WQL, WKH, WKL = (WQK[:, 0], WQK[:, 1], WQK[:, 2], WQK[:, 3])
            TABS = const.tile([128, 2 * QKW + 3 * 512], BF16, tag="TABS")
            nc.sync.dma_start(out=TABS[:], in_=tabs_d.ap())
            COS = TABS[:, 0:QKW]
            SIN = TABS[:, QKW:2 * QKW]
            BAND = TABS[:, 2 * QKW:].rearrange("p (v h j) -> p v h j", v=3, h=2)
            if add_mask:
                MF = const.tile([128, NSC, 2, 256], F32, tag="MF")
                nc.sync.dma_start(
                    out=MF[:],
                    in_=maskf_d.ap().rearrange("p (k h w) -> p k h w", k=NSC, h=2))

            # persistent intermediates. QKa/QKb hold rope output zero-padded
            # per head (h0 on partitions 0-63 of QKa, h1 on 64-127 of QKb):
            # score matmuls then contract all 128 partitions, so the h0/h1
            # matmuls share one PE row group and may drain into one PSUM
            # bank (concurrent row-split tiles on one bank wedge the HW).
            VT = sb.tile([128, VTW], BF16, tag="VT")
            QKa = sb.tile([128, NHP * QKW], BF16, tag="QKa")
            QKb = sb.tile([128, NHP * QKW], BF16, tag="QKb")
            AT = sb.tile([128, NHP * S_CORE], BF16, tag="AT")
            PO1 = sb.tile([128, 6 * S_CORE], F32, tag="PO1")
            nc.vector.memset(QKa[64:128, :], 0.0)
            nc.vector.memset(QKb[0:64, :], 0.0)

            _vb = VT[:, :]
            nc.gpsimd.memset(
                AP(_vb.tensor, _vb.offset + 64,
                   [list(_vb.ap[0]), [1152, NSC], [192, NHP], [1, 64]]), 1.0)


            # ---- V^T projection (stationary = X pair, moving = Wv pair) ----
            def vt_unit(sc):
                for hf in range(2):
                    vp = ps_proj.tile([128, 384], F32, tag="proj")
                    i = 0
                    for pr in range(NPR):
                        for (wt, xt) in ((WVH, XH), (WVL, XH), (WVH, XL)):
                            nc.tensor.matmul(
                                vp[:],
                                xt[:, pr, :, sc * 128:(sc + 1) * 128],
                                wt[:, pr, :, hf * 384:(hf + 1) * 384],
                                start=(i == 0), stop=(i == NPR * 3 - 1),
                                perf_mode=DR)
                            i += 1
                    # vp cols are host-ordered [h0 of hp(a,b,c) | h1 of same];
                    # scatter into VT's [V_h0 | ones | V_h1] blocks.
                    dst = AP(_vb.tensor,
                             _vb.offset + sc * 1152 + hf * 576,
                             [list(_vb.ap[0]), [128, 2], [192, 3], [1, 64]])
                    nc.scalar.activation(
                        dst,
                        vp[:].rearrange("p (h i j) -> p h i j", h=2, i=3),
                        copyf, scale=float(2.0 ** isv))

            # ---- Q/K projection (stationary = W pair, moving = X pair) ----
            def dr3_w(pap, wgt_h, wgt_l, hp, xbase, w):
                i = 0
                for pr in range(NPR):
                    for (wt, xt) in ((wgt_h, XH), (wgt_l, XH), (wgt_h, XL)):
                        nc.tensor.matmul(
                            pap,
                            wt[:, hp, pr, :, :],
                            xt[:, pr, :, xbase:xbase + w],
                            start=(i == 0), stop=(i == NPR * 3 - 1),
                            perf_mode=DR)
                        i += 1

            def proj_hp(hp):
                qsb = ropep.tile([128, QKW], BF16, tag="qsb")
                qp = ps_proj.tile([128, 512], F32, tag="proj")
                dr3_w(qp[:], WQH, WQL, hp, HALO, 512)
                nc.scalar.copy(qsb[:, 0:512], qp[:])
                for half in range(2):
                    kp = ps_proj.tile([128, 320], F32, tag="proj")
                    dr3_w(kp[:], WKH, WKL, hp, half * 320, 320)
                    nc.scalar.copy(
                        qsb[:, 512 + half * 320:512 + (half + 1) * 320],
                        kp[:])
                # rotate_half as +-32 partition swap (sign folded into SIN)
                qrot = ropep.tile([128, QKW], BF16, tag="qrot")
                for blk in range(2):
                    for half in range(2):
                        src = blk * 64 + (1 - half) * 32
                        dst = blk * 64 + half * 32
                        nc.sync.dma_start(
                            out=qrot[dst:dst + 32, :],
                            in_=qsb[src:src + 32, :])
                t1 = ropep.tile([128, QKW], BF16, tag="t1")
                nc.vector.tensor_tensor(t1[:], qsb[:], COS[:], op=mult)
                m2 = ropep.tile([128, QKW], BF16, tag="m2")
                nc.vector.tensor_tensor(m2[:], qrot[:], SIN[:], op=mult)
                nc.vector.tensor_tensor(
                    QKa[0:64, hp * QKW:(hp + 1) * QKW],
                    t1[0:64, :], m2[0:64, :], op=addop)
                nc.vector.tensor_tensor(
                    QKb[64:128, hp * QKW:(hp + 1) * QKW],
                    t1[64:128, :], m2[64:128, :], op=addop)

            # ---- attention stages over units (hp, kc) ----
            def qwin(kc):
                return max(0, kc * 128 - 128), min(S_CORE, kc * 128 + 128)

            def stage_scores(u):
                hp, kc = u["hp"], u["kc"]
                lo, hi = qwin(kc)
                off = 128 if kc == 0 else 0
                s2 = ps_att.tile([128, 512], F32, tag="att",
                                 name=f"s_{hp}_{kc}")
                for h, QKh in enumerate((QKa, QKb)):
                    nc.tensor.matmul(
                        s2[:, h * 256 + off:h * 256 + off + hi - lo],
                        QKh[:, hp * QKW + 512 + kc * 128:
                            hp * QKW + 512 + (kc + 1) * 128],
                        QKh[:, hp * QKW + lo:hp * QKW + hi],
                        start=True, stop=True)
                u["s2"] = s2

            def stage_exp(u):
                hp, kc = u["hp"], u["kc"]
                lo, hi = qwin(kc)
                off = 128 if kc == 0 else 0
                s2 = u["s2"][:].rearrange("p (h j) -> p h j", h=2)
                pe_ = pmp.tile([128, 2, 256], BF16, tag="pe")
                if add_mask:
                    nc.vector.tensor_tensor(
                        s2[:, :, off:off + hi - lo],
                        s2[:, :, off:off + hi - lo],
                        MF[:, kc, :, off:off + hi - lo],
                        op=addop)
                nc.scalar.activation(
                    pe_[:, :, off:off + hi - lo],
                    s2[:, :, off:off + hi - lo], exp)
                u["pe"] = pe_
                del u["s2"]

            def stage_mask(u):
                hp, kc = u["hp"], u["kc"]
                lo, hi = qwin(kc)
                off = 128 if kc == 0 else 0
                var = 0 if kc == 0 else (2 if kc == NSC - 1 else 1)
                pm_ = pmp.tile([128, 2, 256], BF16, tag="pm",
                               name=f"pm_{hp}_{kc}")
                eng = nc.gpsimd if kc in (0, 1, NSC - 1) else nc.vector
                eng.tensor_tensor(
                    pm_[:, :, off:off + hi - lo],
                    u["pe"][:, :, off:off + hi - lo],
                    BAND[:, var, :, off:off + hi - lo],
                    op=mult)
                u["pm"] = pm_
                del u["pe"]

            def vt_stat(kc, hp, h):
                """PV stationary: h0 -> [V_h0 | ones], h1 -> [ones | V_h1]."""
                pos = kc * 1152 + hp * 192 + h * 64
                return VT[:, pos:pos + 128]

            def stage_pv(u, prev_pm):
                hp, kc = u["hp"], u["kc"]
                qb = kc - 1
                if qb == 0:
                    o2s[hp] = [ps_o.tile([128, 512], F32, tag="o",
                                         name=f"o2_{hp}_{h}") for h in range(2)]
                for h in range(2):
                    o2 = o2s[hp][h]
                    nc.tensor.matmul(
                        o2[:, qb * 128:(qb + 1) * 128],
                        vt_stat(qb, hp, h),
                        prev_pm[:, h, 128:256],
                        start=True, stop=False)
                    nc.tensor.matmul(
                        o2[:, qb * 128:(qb + 1) * 128],
                        vt_stat(qb + 1, hp, h),
                        u["pm"][:, h, 0:128],
                        start=False, stop=True)

            def finish_hp(hp):
                # o2[h0]: attn @ 0-63, den @ 64-127 (replicated)
                # o2[h1]: den @ 0-63 (replicated), attn @ 64-127
                # Engines cannot cross partitions, so hop the replicated den
                # rows to SBUF (ACT copy, aligned) and DMA them onto the
                # numerator's partitions, then divide on DVE.
                o2a, o2b = o2s[hp]
                dsb = denp.tile([128, 512], BF16, tag="dsb")
                with nc.allow_low_precision(reason="softmax recip in bf16"):
                    nc.vector.reciprocal(dsb[64:65, :], o2a[64:65, :])
                    nc.vector.reciprocal(dsb[0:1, :], o2b[0:1, :])
                # replicate 1/den onto the numerator's partitions. HW
                # partition_broadcast needs src AND dst at partition 0, so
                # hop den0 (at partition 64) via a 1-row DMA and broadcast
                # to full-height tiles, reading the needed half.
                trow = denp.tile([1, 512], BF16, tag="trow")
                nc.sync.dma_start(out=trow[0:1, :], in_=dsb[64:65, :])
                rda = denp.tile([128, 512], BF16, tag="rda")
                nc.gpsimd.partition_broadcast(rda[:, :], trow[0:1, :])
                rdb = denp.tile([128, 512], BF16, tag="rdb")
                nc.gpsimd.partition_broadcast(rdb[:, :], dsb[0:1, :])
                nc.vector.tensor_tensor(
                    AT[0:64, hp * S_CORE:(hp + 1) * S_CORE],
                    o2a[0:64, :], rda[0:64, :], op=mult)
                nc.vector.tensor_tensor(
                    AT[64:128, hp * S_CORE:(hp + 1) * S_CORE],
                    o2b[64:128, :], rdb[64:128, :], op=mult)
                del o2s[hp]

            o2s = {}

            def outproj_part1():
                for oc in range(6):
                    ops = ps_proj.tile([128, S_CORE], F32, tag="proj")
                    for k in range(5):
                        nc.tensor.matmul(
                            ops[:],
                            WOT[:, k, oc, :],
                            AT[:, k * S_CORE:(k + 1) * S_CORE],
                            start=(k == 0), stop=(k == 4))
                    nc.scalar.copy(PO1[:, oc * S_CORE:(oc + 1) * S_CORE], ops[:])

            # ---- schedule ----
            ks = int(os.environ.get("KSTAGE", "0") or 0)
            vt_unit(0)
            vt_unit(1)
            proj_hp(0)
            vt_unit(2)
            vt_unit(3)
            proj_hp(1)
            vt_unit(4)
            if ks == 1:
                for hp in range(2, NHP):
                    proj_hp(hp)
            if ks == 0 or ks >= 2:
                WOT = sb.tile([128, NHP, 6, 128], BF16, tag="WOT")
                nc.sync.dma_start(
                    out=WOT[:],
                    in_=wot_d.ap().rearrange("p (k o m) -> p k o m", k=NHP, o=6))

                units = [{"hp": hp, "kc": kc}
                         for hp in range(NHP) for kc in range(NSC)]
                NU = len(units)
                stages = [stage_scores, stage_exp, stage_mask]
                if ks == 21:
                    stages = [stage_scores]
                elif ks == 22:
                    stages = [stage_scores, stage_exp]
                ND = len(stages)
                if ks == 25:
                    units = []
                    NU = 0
                    for hp in range(2, NHP):
                        proj_hp(hp)
                for step in range(NU + ND):
                    if step < NU and units[step]["kc"] == 0:
                        hp = units[step]["hp"]
                        if 2 <= hp + 2 < NHP:
                            proj_hp(hp + 2)
                    for k in range(ND - 1, -1, -1):
                        idx = step - k
                        if 0 <= idx < NU:
                            stages[k](units[idx])
                    # pv for the unit whose mask just completed
                    ipv = step - (ND - 1)
                    if 0 <= ipv < NU and units[ipv]["kc"] > 0:
                        if ks in (0, 3):
                            stage_pv(units[ipv], units[ipv - 1]["pm"])
                            if units[ipv]["kc"] == NSC - 1:
                                if ks == 0:
                                    finish_hp(units[ipv]["hp"])
                                    if units[ipv]["hp"] == 4:
                                        outproj_part1()
                                else:
                                    del o2s[units[ipv]["hp"]]

            if ks == 0:
                # ---- output projection tail: hp-4/5 chunks + combine ----
                for oc in range(6):
                    ops = ps_proj.tile([128, S_CORE], F32, tag="proj")
                    nc.tensor.matmul(
                        ops[:], WOT[:, 5, oc, :],
                        AT[:, 5 * S_CORE:6 * S_CORE],
                        start=True, stop=True)
                    ot = outp.tile([128, S_CORE], BF16, tag="ot")
                    nc.vector.scalar_tensor_tensor(
                        out=ot[:], in0=ops[:], scalar=1.0,
                        in1=PO1[:, oc * S_CORE:(oc + 1) * S_CORE],
                        op0=mult, op1=addop)
                    nc.sync.dma_start(
                        out=out_d.ap()[:, oc * S_CORE:(oc + 1) * S_CORE],
                        in_=ot[:])
            else:
                for oc in range(6):
                    ot = outp.tile([128, S_CORE], BF16, tag="ot")
                    nc.vector.tensor_copy(
                        ot[:], QKa[:, oc * 512:(oc + 1) * 512])
                    nc.sync.dma_start(
                        out=out_d.ap()[:, oc * S_CORE:(oc + 1) * S_CORE],
                        in_=ot[:])

    nc.compile()
    return nc


def get_program(add_mask: bool, reps: int = 1, isv: int = 0):
    key = (add_mask, reps, isv)
    if key not in _BUILD_CACHE:
        _BUILD_CACHE[key] = _build(add_mask, isv, reps)
    return _BUILD_CACHE[key]


def _pow2(std):
    return float(2.0 ** np.round(np.log2(1.0 / (std + 1e-30))))


def _fp8_split(a):
    hi = a.astype(ml_dtypes.float8_e4m3fn)
    lo = (a - hi.astype(np.float32)).astype(ml_dtypes.float8_e4m3fn)
    return hi, lo


def _pack_pairs(a, width):
    """[768, width] -> [128, NPR, 2, width] with row = pr*256 + two*128 + p."""
    return np.ascontiguousarray(
        np.asarray(a).reshape(NPR, 2, 128, width).transpose(2, 0, 1, 3))


def prep_core_inputs(core, xs, pos, am, qkv_weight, out_weight, add_mask,
                     scales):
    sX, sQ, sK, sV = scales
    start = S_CORE * core - HALO
    idx = np.arange(start, start + SPAN)
    valid = (idx >= 0) & (idx < SEQ)

    Xs = np.zeros((HID, SPAN), np.float32)
    Xs[:, valid] = xs[:, idx[valid]]
    xhi, xlo = _fp8_split(Xs * sX)

    # rope tables: q part positions [HALO, HALO+512), k part [0, SPAN)
    pspan = np.zeros((SPAN,), np.float32)
    pspan[valid] = pos[idx[valid]]
    invf = (1.0 / (10000.0 ** (np.arange(0, DH, 2, dtype=np.float32)
                               / np.float32(DH)))).astype(np.float32)
    f = pspan[None, :] * invf[:, None]          # [32, SPAN]
    cos64 = np.tile(np.cos(f), (2, 1))          # [64, SPAN]
    sin64 = np.tile(np.sin(f), (2, 1))
    sgn = np.where(np.arange(DH) < DH // 2, -1.0, 1.0).astype(np.float32)
    sin64 = sin64 * sgn[:, None]
    cos128 = np.tile(cos64, (2, 1))             # [128, SPAN]
    sin128 = np.tile(sin64, (2, 1))
    iq = 1.0 / (sX * sQ)
    ik = 1.0 / (sX * sK)
    cosqk = np.concatenate(
        [cos128[:, HALO:HALO + S_CORE] * iq, cos128 * ik], 1)
    sinqk = np.concatenate(
        [sin128[:, HALO:HALO + S_CORE] * iq, sin128 * ik], 1)

    # band masks in [S(i), (h, j)] layout; in-band iff j-128 <= i <= j, plus
    # global key-existence at the sequence edges (kc=0 / kc=4 variants).
    i = np.arange(128)[:, None]
    j = np.arange(256)[None, :]
    band = ((j - 128 <= i) & (i <= j)).astype(np.float32)  # [128, 256]
    gkey0 = start + np.arange(128)
    gkey4 = start + 4 * 128 + np.arange(128)
    v0 = ((gkey0 >= 0) & (gkey0 < SEQ)).astype(np.float32)[:, None]
    v4 = ((gkey4 >= 0) & (gkey4 < SEQ)).astype(np.float32)[:, None]
    bandm = np.zeros((128, 3, 2, 256), np.float32)
    for h in range(2):
        bandm[:, 0, h] = band * v0
        bandm[:, 1, h] = band
        bandm[:, 2, h] = band * v4

    wq = qkv_weight[0:HID] * np.float32(DH ** -0.5)
    wk = qkv_weight[HID:2 * HID]
    wv = qkv_weight[2 * HID:3 * HID]

    def pack_w_hp(w, s):
        wt = np.ascontiguousarray(w.T) * s       # [768c, 768o]
        hi, lo = _fp8_split(wt)

        def hp_major(p):
            return np.ascontiguousarray(
                _pack_pairs(p, HID).reshape(128, NPR, 2, NHP, 128)
                .transpose(0, 3, 1, 2, 4)).reshape(128, NHP * NPR * 2 * 128)

        return hp_major(hi), hp_major(lo)

    wqh, wql = pack_w_hp(wq, sQ)
    wkh, wkl = pack_w_hp(wk, sK)
    # reorder wv columns so each vp half is [h0 of 3 hps | h1 of same]
    head_order = [hf * 6 + t for hf in range(2) for t in (0, 2, 4, 1, 3, 5)]
    col_perm = np.concatenate([np.arange(hg * 64, hg * 64 + 64)
                               for hg in head_order])
    vt_cols = np.ascontiguousarray(wv.T[:, col_perm]) * sV
    vhi, vlo = _fp8_split(vt_cols)
    wvh = _pack_pairs(vhi, HID).reshape(128, NPR * 2 * HID)
    wvl = _pack_pairs(vlo, HID).reshape(128, NPR * 2 * HID)

    # out_weight stationary: wot[p, k(hp), oc, j] = out_weight[oc*128+j, k*128+p]
    wot = np.ascontiguousarray(
        out_weight.reshape(6, 128, NHP, 128).transpose(3, 2, 0, 1)
    ).astype(ml_dtypes.bfloat16)

    xh_p = _pack_pairs(xhi, SPAN).reshape(128, NPR * 2 * SPAN)
    xl_p = _pack_pairs(xlo, SPAN).reshape(128, NPR * 2 * SPAN)
    in_map = {
        "xhl": np.ascontiguousarray(np.concatenate([xh_p, xl_p], 1)),
        "wqd": np.ascontiguousarray(np.concatenate([wqh, wql], 1)),
        "wkd": np.ascontiguousarray(np.concatenate([wkh, wkl], 1)),
        "wvhl": np.ascontiguousarray(np.concatenate([wvh, wvl], 1)),
        "wot": np.ascontiguousarray(wot.reshape(128, NHP * 6 * 128)),
        "tabs": np.ascontiguousarray(np.concatenate(
            [cosqk.astype(ml_dtypes.bfloat16),
             sinqk.astype(ml_dtypes.bfloat16),
             bandm.reshape(128, 3 * 512).astype(ml_dtypes.bfloat16)], 1)),
    }
    if add_mask:
        mf = np.zeros((128, NSC, 2, 256), np.float32)
        for kc in range(NSC):
            qlo = max(0, kc * 128 - 128)
            qhi = min(S_CORE, kc * 128 + 128)
            off = 128 if kc == 0 else 0
            gq = S_CORE * core + np.arange(qlo, qhi)
            gk = start + kc * 128 + np.arange(128)
            kvalid = (gk >= 0) & (gk < SEQ)
            sub = np.zeros((128, qhi - qlo), np.float32)
            sub[kvalid, :] = am[np.ix_(gq, gk[kvalid])].T
            for h in range(2):
                mf[:, kc, h, off:off + qhi - qlo] = sub
        in_map["maskf"] = np.ascontiguousarray(mf.reshape(128, NSC * 512))
    return in_map


def prep_all_inputs(x, position_ids, attention_mask, qkv_weight, out_weight):
    xs = np.asarray(x, dtype=np.float32)[0, :, 0, :]
    pos = np.asarray(position_ids)[0].astype(np.float32)
    am = np.asarray(attention_mask, dtype=np.float32)[0, 0]
    qkv_w = np.asarray(qkv_weight, dtype=np.float32)
    out_w = np.asarray(out_weight, dtype=np.float32)
    add_mask = bool(np.any(am))
    sX = _pow2(xs.std())
    sQ = _pow2((qkv_w[0:HID] * np.float32(DH ** -0.5)).std())
    sK = _pow2(qkv_w[HID:2 * HID].std())
    sV = _pow2(qkv_w[2 * HID:3 * HID].std())
    scales = (sX, sQ, sK, sV)
    isv = int(np.round(np.log2(1.0 / (sX * sV))))
    in_maps = [
        prep_core_inputs(c, xs, pos, am, qkv_w, out_w, add_mask, scales)
        for c in range(N_CORES)
    ]
    return in_maps, add_mask, isv


def assemble_output(results):
    cols = []
    for c in range(N_CORES):
        o = np.asarray(results[c]["out"]).astype(np.float32)  # [128, 6*512]
        cols.append(
            o.reshape(128, 6, S_CORE).transpose(1, 0, 2).reshape(HID, S_CORE))
    full = np.concatenate(cols, axis=1)
    return np.ascontiguousarray(full.reshape(1, HID, 1, SEQ), dtype=np.float32)


def kernel(**inputs):
    in_maps, add_mask, isv = prep_all_inputs(
        inputs["x"], inputs["position_ids"], inputs["attention_mask"],
        inputs["qkv_weight"], inputs["out_weight"])
    nc = get_program(add_mask, isv=isv)
    res = run_bass_kernel_spmd(nc, in_maps, core_ids=list(range(N_CORES)))
    return assemble_output(res.results)
